# revision 2
# baseline (speedup 1.0000x reference)
"""Trainium2 Bass kernel for DecoupledMVRowSelfAttnProcessor (bs=6, seq=1024,
C=1280, 20 heads, 6 views, row-wise MV attention). Self-contained 8-core SPMD.

Sharding (c = 4g + j; g in {0,1} batch-group of 3 views, j in {0..3} head-group):
  - base/ref attention: HEAD-SHARDED. Core (g,j) projects Q/K/V for heads
    5j..5j+5 (320 cols) over its 3 batches (3072 tokens) from host-fed bf16
    X^T / refX^T, runs 5x3 flash-style attentions ([V|1] row-sum trick, exp on
    ACT, normalize via reciprocal+partition_broadcast), then a PARTIAL
    out-projection (Wout rows 320j..) for all 3072 tokens. Partials are summed
    and token-scattered by 3 column-split 4-wide ReduceScatters that overlap
    the out-projection compute.
  - MV row attention: ROW-SHARDED. Core c owns image rows {c,c+8,c+16,c+24};
    host feeds the 768 X^T columns of those rows. Full-C Q/K/V projections,
    4 row-attentions (192 kv), full MV out-projection, then one perfectly
    uniform 8-wide AllToAll (96 rows per core pair) routes 32-token blocks to
    their token-owners; launched first so it overlaps all base/ref work.
  - K/Q/V stay SBUF-resident (no K/V gathers at all vs the v1 kernel's 8 big
    AllGathers); attention output overwrites the consumed Q slices; ref K/V
    overwrite base K/V buffers after base attention (WAR-ordered by Tile).


v3 + scheduling fixes:
  - MV weights as full [128,10,C] tiles (bufs=2) again; OmT aliases xm's buffer.
  - wq..wqr loads get a pool allocated before pMw so no space-WAR delays them.
  - KrT/Vr alias KbT/Vb buffers (ref proj naturally follows base attention);
    rg prefetches into its own buffer during base projections.
  - omL writes / omG reads issued from SP (HWDGE) instead of gpsimd SWDGE
    (2.8us/desc-gen there was serializing the tail).
  - software-pipelined attention: scores/exp of unit u+1 issue before AV of
    unit u, hiding exp latency from the PE stream (both MV and base/ref).
"""
import sys
sys.path.insert(0, '/opt/trn_rl_repo')
import contextlib
import math
import os
import numpy as np

import concourse.bass as bass
import concourse.mybir as mybir
from concourse import bacc
from concourse.tile import TileContext
from concourse.bass_utils import run_bass_kernel_spmd

f32 = mybir.dt.float32
bf16 = mybir.dt.bfloat16
u32 = mybir.dt.uint32
AF = mybir.ActivationFunctionType
ALU = mybir.AluOpType

NCORES = 8
BS, SEQ, C = 6, 1024, 1280
H, HD, NV = 20, 64, 6
IH = IW = 32
TOK = BS * SEQ
TPC = TOK // NCORES       # 768
GTOK = 3 * SEQ            # 3072
HPC = H // 4              # 5
HC = HPC * HD             # 320
NCI = C // 128            # 10
LKV = NV * IW             # 192
SCALE = 1.0 / math.sqrt(HD)
PLAN_LEN = 64
CCH = ((0, 512), (512, 1024), (1024, 1280))
HCT = ((0, 128), (128, 256), (256, 320))

_CACHE = {}


def _build():
    nc = bacc.Bacc("TRN2", target_bir_lowering=False, debug=False, num_devices=NCORES)

    xTg = nc.declare_dram_parameter("xTg", [C, GTOK], bf16, isOutput=False)
    rTg = nc.declare_dram_parameter("rTg", [C, GTOK], bf16, isOutput=False)
    xTmv = nc.declare_dram_parameter("xTmv", [C, TPC], bf16, isOutput=False)
    res = nc.declare_dram_parameter("res", [TPC, C], f32, isOutput=False)
    bsum = nc.declare_dram_parameter("bsum", [1, C], f32, isOutput=False)
    plan = nc.declare_dram_parameter("plan", [1, PLAN_LEN], u32, isOutput=False)
    WS = {}
    for n in ("wq", "wk", "wv", "wqr", "wkr", "wvr"):
        WS[n] = nc.declare_dram_parameter(n, [C, HC], bf16, isOutput=False)
    for n in ("wo", "wor"):
        WS[n] = nc.declare_dram_parameter(n, [HC, C], bf16, isOutput=False)
    for n in ("wqm", "wkm", "wvm", "wom"):
        WS[n] = nc.declare_dram_parameter(n, [C, C], bf16, isOutput=False)
    out = nc.declare_dram_parameter("out_shard", [TPC, C], f32, isOutput=True)

    with TileContext(nc) as tc, contextlib.ExitStack() as topstack:
        const = topstack.enter_context(tc.tile_pool(name="const", bufs=1))
        dram = topstack.enter_context(tc.tile_pool(name="dram", bufs=1, space="DRAM"))

        plan_sb = const.tile([1, PLAN_LEN], u32)
        nc.gpsimd.dma_start(plan_sb[:], plan[:])
        bsum_sb = const.tile([1, C], f32)
        nc.sync.dma_start(bsum_sb[:], bsum[:])
        bias_bc = const.tile([128, C], f32)
        nc.gpsimd.partition_broadcast(bias_bc[:], bsum_sb[:])

        def plan_reg(eng, idx, max_val):
            tmp = eng.alloc_register(f"plan_{idx}_{nc.next_id()}")
            eng.reg_load(tmp, plan_sb[0:1, idx:idx + 1])
            return eng.snap(tmp, donate=True, min_val=0, max_val=max_val)

        omL = dram.tile([TPC, C], bf16, tag="omL")
        omG = dram.tile([TPC, C], bf16, tag="omG")
        prL = [dram.tile([GTOK, c1 - c0], bf16, name=f"prL{k}", tag=f"prL{k}")
               for k, (c0, c1) in enumerate(CCH)]
        rsG = [dram.tile([TPC, c1 - c0], bf16, name=f"rsG{k}", tag=f"rsG{k}")
               for k, (c0, c1) in enumerate(CCH)]
        G8 = [list(range(NCORES))]
        GRS = [[0, 1, 2, 3], [4, 5, 6, 7]]

        def copyback(dst_ap, src_ap, idx):
            if idx % 2:
                nc.vector.tensor_copy(dst_ap, src_ap)
            else:
                nc.scalar.copy(dst_ap, src_ap)

        for _rep in range(int(os.environ.get('BASS_KERNEL_REPS', '1'))):
          with contextlib.ExitStack() as stack:
            # pools whose placement matters: small weight pool FIRST so it
            # never lands on MV-phase space (avoids load-delaying WARs)
            pPw = stack.enter_context(tc.tile_pool(name="pPw", bufs=1))
            pPx = stack.enter_context(tc.tile_pool(name="pPx", bufs=1))

            # =================== MV phase ====================================
            with tc.tile_pool(name="pXM", bufs=1) as pXM, \
                 tc.tile_pool(name="pMx", bufs=1) as pMx, \
                 tc.tile_pool(name="pMw", bufs=2) as pMw:

                xm = pXM.tile([128, NCI, TPC], bf16, tag="xm")
                nc.scalar.dma_start(xm[:], xTmv[:].rearrange("(ci p) t -> p ci t", p=128))

                KmT = pMx.tile([128, NCI, TPC], bf16, tag="KmT")
                QmT = pMx.tile([128, NCI, TPC], bf16, tag="QmT")
                Vm = pMx.tile([128, 8, H, HD + 1], bf16, tag="Vm")

                def load_wm(wname):
                    w = pMw.tile([128, NCI, C], bf16, name=f"w_{wname}", tag="w_m")
                    nc.sync.dma_start(
                        w[:], WS[wname][:].rearrange("(ci p) k -> p ci k", p=128))
                    return w

                with tc.tile_pool(name="psM", bufs=3, space="PSUM") as psM:
                    for wname, dstT in (("wkm", KmT), ("wqm", QmT)):
                        w = load_wm(wname)
                        for co in range(NCI):
                            for k, (c0, c1) in enumerate(((0, 512), (512, 768))):
                                ps = psM.tile([128, 512], f32, tag="psMp")
                                for ci in range(NCI):
                                    nc.tensor.matmul(
                                        ps[:, :c1 - c0],
                                        w[:, ci, co * 128:(co + 1) * 128],
                                        xm[:, ci, c0:c1],
                                        start=(ci == 0), stop=(ci == NCI - 1))
                                copyback(dstT[:, co, c0:c1], ps[:, :c1 - c0], co + k)
                    wvm = load_wm("wvm")
                    for sub in range(8):
                        tok0 = (sub // 2) * LKV + (sub % 2) * 128
                        ntok = 128 if sub % 2 == 0 else 64
                        nc.any.memset(Vm[:, sub, :, HD:HD + 1], 1.0)
                        for k, (c0, c1) in enumerate(CCH):
                            ps = psM.tile([128, 512], f32, tag="psMp")
                            for ci in range(NCI):
                                nc.tensor.matmul(
                                    ps[:ntok, :c1 - c0],
                                    xm[:, ci, tok0:tok0 + ntok],
                                    wvm[:, ci, c0:c1],
                                    start=(ci == 0), stop=(ci == NCI - 1))
                            copyback(Vm[0:ntok, sub, c0 // HD:c1 // HD, 0:HD],
                                     ps[:ntok, :c1 - c0]
                                     .rearrange("p (h d) -> p h d", d=HD), k)

                # OmT reuses xm's buffer (xm is dead after the V projection)
                OmT = pXM.tile([128, NCI, TPC], bf16, name="OmT", tag="xm")

                # prefetch xg (base phase) while MV attention runs
                xg = pPx.tile([128, NCI, GTOK], bf16, tag="xg")
                nc.scalar.dma_start(xg[:], xTg[:].rearrange("(ci p) t -> p ci t", p=128))

                # 4 row-attentions x 20 heads, software-pipelined depth 1
                with tc.tile_pool(name="pMa", bufs=4) as pMa, \
                     tc.tile_pool(name="psMa", bufs=4, space="PSUM") as psMa:
                    mstate = {}

                    def m_scores(i):
                        rl, h = divmod(i, H)
                        kc = rl * LKV
                        ht, hr = h // 2, (h % 2) * 64
                        s_ps = psMa.tile([128, 2, LKV], f32, tag="ms")
                        nc.tensor.matmul(s_ps[:, 0, :],
                                         KmT[hr:hr + 64, ht, kc:kc + 128],
                                         QmT[hr:hr + 64, ht, kc:kc + LKV],
                                         start=True, stop=True)
                        nc.tensor.matmul(s_ps[0:64, 1, :],
                                         KmT[hr:hr + 64, ht, kc + 128:kc + LKV],
                                         QmT[hr:hr + 64, ht, kc:kc + LKV],
                                         start=True, stop=True)
                        a_sb = pMa.tile([128, 2, LKV], bf16, tag="ma")
                        nc.scalar.activation(a_sb[:, 0, :], s_ps[:, 0, :],
                                             AF.Exp, scale=SCALE)
                        nc.scalar.activation(a_sb[0:64, 1, :], s_ps[0:64, 1, :],
                                             AF.Exp, scale=SCALE)
                        mstate[i] = a_sb

                    def m_finish(i):
                        rl, h = divmod(i, H)
                        kc = rl * LKV
                        ht, hr = h // 2, (h % 2) * 64
                        a_sb = mstate.pop(i)
                        o_ps = psMa.tile([HD + 1, LKV], f32, tag="mo")
                        nc.tensor.matmul(o_ps[:], Vm[:, 2 * rl, h, :],
                                         a_sb[:, 0, :], start=True, stop=False)
                        nc.tensor.matmul(o_ps[:], Vm[0:64, 2 * rl + 1, h, :],
                                         a_sb[0:64, 1, :], start=False, stop=True)
                        rec = pMa.tile([1, LKV], f32, tag="mrec")
                        nc.vector.reciprocal(rec[:], o_ps[HD:HD + 1, :])
                        rep = pMa.tile([HD, LKV], f32, tag="mrep")
                        nc.gpsimd.partition_broadcast(rep[:], rec[:])
                        nc.vector.tensor_tensor(
                            out=OmT[hr:hr + 64, ht, kc:kc + LKV],
                            in0=o_ps[0:HD, :], in1=rep[:], op=ALU.mult)

                    m_scores(0)
                    for i in range(4 * H):
                        if i + 1 < 4 * H:
                            m_scores(i + 1)
                        m_finish(i)

                # MV out-projection -> omL (dest-ordered rows, via SP queue)
                wom = load_wm("wom")
                with tc.tile_pool(name="pMo", bufs=3) as pMo, \
                     tc.tile_pool(name="psMo", bufs=3, space="PSUM") as psMo:
                    for t in range(TPC // 128):
                        stg = pMo.tile([128, C], bf16, tag="mo_stg")
                        for k, (c0, c1) in enumerate(CCH):
                            ps = psMo.tile([128, 512], f32, tag="psMo")
                            for ci in range(NCI):
                                nc.tensor.matmul(
                                    ps[:, :c1 - c0],
                                    OmT[:, ci, t * 128:(t + 1) * 128],
                                    wom[:, ci, c0:c1],
                                    start=(ci == 0), stop=(ci == NCI - 1))
                            copyback(stg[:, c0:c1], ps[:, :c1 - c0], k)
                        for q in range(4):
                            mo = plan_reg(nc.sync, t * 4 + q, TPC - IW)
                            nc.sync.dma_start(omL[bass.ds(mo, IW), :],
                                              stg[q * IW:(q + 1) * IW, :])
                nc.gpsimd.collective_compute(
                    "AllToAll", ALU.bypass, replica_groups=G8,
                    ins=[omL[:].opt()], outs=[omG[:].opt()])

            # =================== base/ref phase ==============================
            resid = stack.enter_context(tc.tile_pool(name="resident", bufs=1))

            def hc_tiles(name, tagbase=None):
                tb = tagbase or name
                return [resid.tile([128, GTOK], bf16, name=f"{name}0", tag=f"{tb}0"),
                        resid.tile([128, GTOK], bf16, name=f"{name}1", tag=f"{tb}1"),
                        resid.tile([64, GTOK], bf16, name=f"{name}2", tag=f"{tb}2")]

            def proj_T(psP, wname, xsrc, dstT):
                w = pPw.tile([128, NCI, HC], bf16, name=f"w_{wname}", tag="w_t")
                nc.sync.dma_start(w[:], WS[wname][:].rearrange("(ci p) k -> p ci k", p=128))
                for o, (h0, h1) in enumerate(HCT):
                    for cc in range(GTOK // 512):
                        ps = psP.tile([128, 512], f32, tag="psP")
                        for ci in range(NCI):
                            nc.tensor.matmul(ps[:h1 - h0, :], w[:, ci, h0:h1],
                                             xsrc[:, ci, cc * 512:(cc + 1) * 512],
                                             start=(ci == 0), stop=(ci == NCI - 1))
                        copyback(dstT[o][:, cc * 512:(cc + 1) * 512], ps[:h1 - h0, :], cc)

            def proj_V(psP, wname, xsrc, dstV):
                w = pPw.tile([128, NCI, HC], bf16, name=f"w_{wname}", tag="w_t")
                nc.sync.dma_start(w[:], WS[wname][:].rearrange("(ci p) k -> p ci k", p=128))
                for t in range(GTOK // 128):
                    nc.any.memset(dstV[:, t, :, HD:HD + 1], 1.0)
                    ps = psP.tile([128, 512], f32, tag="psP")
                    for ci in range(NCI):
                        nc.tensor.matmul(ps[:, :HC], xsrc[:, ci, t * 128:(t + 1) * 128],
                                         w[:, ci, :], start=(ci == 0), stop=(ci == NCI - 1))
                    copyback(dstV[:, t, :, 0:HD],
                             ps[:, :HC].rearrange("p (h d) -> p h d", d=HD), t)

            def attention(KT, QT, OT, V, psS, psO, pA):
                units = [(h, b, qc) for h in range(HPC) for b in range(3)
                         for qc in range(2)]
                astate = {}

                def u_scores(u):
                    h, b, qc = units[u]
                    ht, hr = h // 2, (h % 2) * 64
                    k0 = b * SEQ
                    q0 = b * SEQ + qc * 512
                    a_sb = {}
                    for gg in range(4):
                        s_ps = psS.tile([128, 2, 512], f32, tag="s_ps")
                        for kk in range(2):
                            kt = k0 + (gg * 2 + kk) * 128
                            nc.tensor.matmul(
                                s_ps[:, kk, :],
                                KT[ht][hr:hr + 64, kt:kt + 128],
                                QT[ht][hr:hr + 64, q0:q0 + 512],
                                start=True, stop=True)
                        ab = pA.tile([128, 2, 512], bf16, tag=f"a_sb{gg}")
                        nc.scalar.activation(
                            ab[:].rearrange("p a b -> p (a b)"),
                            s_ps[:].rearrange("p a b -> p (a b)"),
                            AF.Exp, scale=SCALE)
                        a_sb[gg] = ab
                    astate[u] = a_sb

                def u_finish(u):
                    h, b, qc = units[u]
                    ht, hr = h // 2, (h % 2) * 64
                    k0 = b * SEQ
                    q0 = b * SEQ + qc * 512
                    a_sb = astate.pop(u)
                    o_ps = psO.tile([HD + 1, 512], f32, tag="o_ps")
                    for kt in range(8):
                        vt = (k0 // 128) + kt
                        nc.tensor.matmul(
                            o_ps[:], V[:, vt, h, :],
                            a_sb[kt // 2][:, kt % 2, :],
                            start=(kt == 0), stop=(kt == 7))
                    rec = pA.tile([1, 512], f32, tag="rec")
                    nc.vector.reciprocal(rec[:], o_ps[HD:HD + 1, :])
                    rep = pA.tile([HD, 512], f32, tag="rep")
                    nc.gpsimd.partition_broadcast(rep[:], rec[:])
                    nc.vector.tensor_tensor(
                        out=OT[ht][hr:hr + 64, q0:q0 + 512],
                        in0=o_ps[0:HD, :], in1=rep[:], op=ALU.mult)

                u_scores(0)
                for u in range(len(units)):
                    if u + 1 < len(units):
                        u_scores(u + 1)
                    u_finish(u)

            # base projections (xg resident); rg prefetches concurrently
            QbT = hc_tiles("QbT")
            KbT = hc_tiles("KbT")
            Vb = resid.tile([128, GTOK // 128, HPC, HD + 1], bf16, tag="Vb")
            QrT = hc_tiles("QrT")
            # rg reuses xg's buffer; its load starts when base projections have
            # drained xg and overlaps base attention (ref proj follows it anyway)
            rg = pPx.tile([128, NCI, GTOK], bf16, name="rg", tag="xg")
            nc.scalar.dma_start(rg[:], rTg[:].rearrange("(ci p) t -> p ci t", p=128))
            with tc.tile_pool(name="psP", bufs=4, space="PSUM") as psP:
                proj_T(psP, "wq", xg, QbT)
                proj_T(psP, "wk", xg, KbT)
                proj_V(psP, "wv", xg, Vb)
                proj_T(psP, "wqr", xg, QrT)

            ObT, OrT = QbT, QrT   # attention output overwrites consumed Q slice

            with tc.tile_pool(name="pA", bufs=2) as pA, \
                 tc.tile_pool(name="psS", bufs=2, space="PSUM") as psS, \
                 tc.tile_pool(name="psO", bufs=2, space="PSUM") as psO:
                # ref K/V overwrite base K/V buffers; the WAR on base attention's
                # reads naturally orders ref projections behind base attention.
                KrT = hc_tiles("KrT", tagbase="KbT")
                Vr = resid.tile([128, GTOK // 128, HPC, HD + 1], bf16,
                                name="Vr", tag="Vb")
                attention(KbT, QbT, ObT, Vb, psS, psO, pA)
                with tc.tile_pool(name="psP2", bufs=2, space="PSUM") as psP2:
                    proj_T(psP2, "wkr", rg, KrT)
                    proj_V(psP2, "wvr", rg, Vr)
                attention(KrT, QrT, OrT, Vr, psS, psO, pA)

            # =================== partial out-proj + RS + final ================
            with tc.tile_pool(name="pC", bufs=3) as pC, \
                 tc.tile_pool(name="pCw", bufs=1) as pCw, \
                 tc.tile_pool(name="pF", bufs=2) as pF, \
                 tc.tile_pool(name="psC", bufs=3, space="PSUM") as psC:
                wo_sb, wor_sb = [], []
                for nm, dst in (("wo", wo_sb), ("wor", wor_sb)):
                    for o, (h0, h1) in enumerate(HCT):
                        wt = pCw.tile([h1 - h0, C], bf16, tag=f"w_{nm}{o}")
                        nc.sync.dma_start(wt[:], WS[nm][h0:h1, :])
                        dst.append(wt)
                for k, (c0, c1) in enumerate(CCH):
                    cw = c1 - c0
                    for t in range(GTOK // 128):
                        ps = psC.tile([128, 512], f32, tag="psC")
                        first = True
                        for OT, wsb in ((ObT, wo_sb), (OrT, wor_sb)):
                            for o, (h0, h1) in enumerate(HCT):
                                nc.tensor.matmul(ps[:, :cw],
                                                 OT[o][:, t * 128:(t + 1) * 128],
                                                 wsb[o][:, c0:c1],
                                                 start=first, stop=(o == 2 and OT is OrT))
                                first = False
                        stg = pC.tile([128, 512], bf16, tag="c_stg")
                        copyback(stg[:, :cw], ps[:, :cw], t)
                        nc.scalar.dma_start(prL[k][t * 128:(t + 1) * 128, :], stg[:, :cw])
                    nc.gpsimd.collective_compute(
                        "ReduceScatter", ALU.add, replica_groups=GRS,
                        ins=[prL[k][:].opt()], outs=[rsG[k][:].opt()])
                # combines emitted after ALL compute so their RS-gated DMAs
                # never head-of-line-block the compute queues
                for k, (c0, c1) in enumerate(CCH):
                    cw = c1 - c0
                    for t in range(TPC // 128):
                        rs_t = pF.tile([128, 512], bf16, tag="rs_t")
                        nc.scalar.dma_start(rs_t[:, :cw], rsG[k][t * 128:(t + 1) * 128, :])
                        om_t = pF.tile([128, 512], bf16, tag="om_t")
                        for q in range(4):
                            mo = plan_reg(nc.sync, 24 + t * 4 + q, TPC - IW)
                            nc.sync.dma_start(om_t[q * IW:(q + 1) * IW, :cw],
                                              omG[bass.ds(mo, IW), c0:c1])
                        res_t = pF.tile([128, 512], f32, tag="res_t")
                        nc.sync.dma_start(res_t[:, :cw], res[t * 128:(t + 1) * 128, c0:c1])
                        rs32 = pF.tile([128, 512], f32, tag="rs32")
                        nc.vector.tensor_copy(rs32[:, :cw], rs_t[:, :cw])
                        om32 = pF.tile([128, 512], f32, tag="om32")
                        nc.vector.tensor_tensor(out=om32[:, :cw], in0=om_t[:, :cw],
                                                in1=bias_bc[:, c0:c1], op=ALU.add)
                        t1 = pF.tile([128, 512], f32, tag="t1")
                        nc.vector.tensor_tensor(out=t1[:, :cw], in0=rs32[:, :cw],
                                                in1=res_t[:, :cw], op=ALU.add)
                        out_t = pF.tile([128, 512], f32, tag="out_t")
                        nc.vector.tensor_tensor(out=out_t[:, :cw], in0=t1[:, :cw],
                                                in1=om32[:, :cw], op=ALU.add)
                        nc.sync.dma_start(out[t * 128:(t + 1) * 128, c0:c1], out_t[:, :cw])

    nc.compile()
    return nc


def _owner(tok):
    return (tok // GTOK) * 4 + (tok % GTOK) // TPC


def _plans():
    plans = []
    for c in range(NCORES):
        p = np.zeros(PLAN_LEN, dtype=np.uint32)
        dest_slots = {}
        per_dest = {d: [] for d in range(NCORES)}
        for t in range(4):
            for v in range(NV):
                tau0 = v * SEQ + (c + 8 * t) * IW
                per_dest[_owner(tau0)].append((t, v))
        for d in range(NCORES):
            assert len(per_dest[d]) == 3, (c, d, per_dest[d])
            for slot, (t, v) in enumerate(sorted(per_dest[d])):
                dest_slots[(t, v)] = d * 96 + slot * IW
        for t in range(4):
            for v in range(NV):
                p[t * NV + v] = dest_slots[(t, v)]
        T0 = (c // 4) * GTOK + (c % 4) * TPC
        for tt in range(TPC // 128):
            for q in range(4):
                tau0 = T0 + tt * 128 + q * IW
                v, rem = tau0 // SEQ, tau0 % SEQ
                r = rem // IW
                s, t = r % 8, r // 8
                blocks = sorted((t2, v2) for t2 in range(4) for v2 in range(NV)
                                if _owner(v2 * SEQ + (s + 8 * t2) * IW) == c)
                slot = blocks.index((t, v))
                p[24 + tt * 4 + q] = s * 96 + slot * IW
        plans.append(p.reshape(1, PLAN_LEN))
    return plans


def _in_maps(inputs):
    import ml_dtypes
    hid = np.asarray(inputs["hidden_states"], dtype=np.float32).reshape(TOK, C)
    ref = np.asarray(inputs["ref_hidden_states"], dtype=np.float32).reshape(TOK, C)
    bsum = (np.asarray(inputs["bout"]) + np.asarray(inputs["bout_mv"])
            + np.asarray(inputs["bout_ref"])).astype(np.float32).reshape(1, C)
    W = {n: np.asarray(inputs[n], dtype=np.float32) for n in
         ["Wq", "Wk", "Wv", "Wq_mv", "Wk_mv", "Wv_mv",
          "Wq_ref", "Wk_ref", "Wv_ref", "Wout", "Wout_mv", "Wout_ref"]}
    plans = _CACHE.setdefault("plans", _plans())
    in_maps = []
    for c in range(NCORES):
        g, j = c // 4, c % 4
        gs = slice(g * GTOK, (g + 1) * GTOK)
        hs = slice(j * HC, (j + 1) * HC)
        rows = [c, c + 8, c + 16, c + 24]
        cols = np.concatenate([np.arange(v * SEQ + r * IW, v * SEQ + (r + 1) * IW)
                               for r in rows for v in range(NV)])
        m = {
            "xTg": hid[gs].T, "rTg": ref[gs].T, "xTmv": hid[cols].T,
            "res": np.ascontiguousarray(hid[c * TPC:(c + 1) * TPC]),
            "bsum": bsum, "plan": plans[c],
            "wq": W["Wq"][:, hs], "wk": W["Wk"][:, hs], "wv": W["Wv"][:, hs],
            "wqr": W["Wq_ref"][:, hs], "wkr": W["Wk_ref"][:, hs], "wvr": W["Wv_ref"][:, hs],
            "wo": W["Wout"][hs, :], "wor": W["Wout_ref"][hs, :],
            "wqm": W["Wq_mv"], "wkm": W["Wk_mv"], "wvm": W["Wv_mv"], "wom": W["Wout_mv"],
        }
        for k in ("xTg", "rTg", "xTmv", "wq", "wk", "wv", "wqr", "wkr", "wvr",
                  "wo", "wor", "wqm", "wkm", "wvm", "wom"):
            m[k] = np.ascontiguousarray(m[k]).astype(ml_dtypes.bfloat16)
        in_maps.append(m)
    return in_maps


def kernel(**inputs):
    if "nc" not in _CACHE:
        _CACHE["nc"] = _build()
    nc = _CACHE["nc"]
    in_maps = _in_maps(inputs)
    res = run_bass_kernel_spmd(nc, in_maps, list(range(NCORES)))
    full = np.concatenate([res.results[c]["out_shard"] for c in range(NCORES)], axis=0)
    return full.reshape(BS, SEQ, C)


if __name__ == "__main__":
    _build()
    print("BUILD OK")
